# revision 2
# baseline (speedup 1.0000x reference)
"""Trainium2 kernel for nn_HashCodeAwareLogits - TensorEngine dot-product design.

The 512MB bucket table is row-sharded across 8 cores (8192 rows each, bf16).
Each (pos, hash) instance is routed to the core owning its bucket row.

Per core, distinct rows are deduped and bin-packed into "sets" of <=8 rows
and <=32 instances. 16 sets (128 rows) are fetched per dma_gather with
transpose=True, which lands row element (a*64+e) at partition (a%2)*64+e,
free offset (c=a//2, i=row-slot) — i.e. the contraction dim e sits on the
partition axis. One 128x128x64 matmul per set then computes every dot
product out[(c, il), (j, t)] = sum_e row_il[2c+j, e] * tvw_t[e] on the
TensorEngine with fp32 PSUM accumulation. ScalarE drains PSUM (cast to
bf16), and the host extracts each instance's 32 logits and adds the two
hash contributions.
"""

import math

import ml_dtypes
import numpy as np

import concourse.bass as bass
import concourse.mybir as mybir
from concourse import bacc
from concourse import library_config
from concourse.bass_utils import run_bass_kernel_spmd
from concourse.tile import TileContext

PRIME = (1 << 31) - 1
N_DIGITS = 16
N_ARY = 32
EMB = 64
NUM_EMB = 100000
NUM_BUCKETS = 65536
NUM_HASHES = 2
N_CORES = 8
ROWS_PER_CORE = NUM_BUCKETS // N_CORES  # 8192
P = 128
ROW_W = N_ARY * EMB  # 2048

S_SLOTS = 32          # instance slots per set
ROWS_PER_SET = 8
GATHER_ROWS = 128     # rows per dma_gather (multiple of 128)
SETS_PER_GATHER = GATHER_ROWS // ROWS_PER_SET  # 16

_rng = np.random.RandomState(42)
SEQ_A = _rng.randint(1, PRIME, size=(N_DIGITS,)).astype(np.int64)
HASH_A = _rng.randint(1, PRIME, size=(NUM_HASHES,)).astype(np.int64)
HASH_B = _rng.randint(0, PRIME, size=(NUM_HASHES,)).astype(np.int64)

TRACE = False
LAST_RESULT = None

_PROGRAM_CACHE = {}


def _ensure_ntff_hook():
    import sys
    import types

    if "antenv.axon_hooks" in sys.modules:
        return
    try:
        sys.path.insert(0, "/root/.axon_site/trn_agent_boot")
        import trn_boot  # type: ignore

        hook = trn_boot._ntff_profile_via_ctypes("/opt/axon/libaxon_pjrt.so")
    except Exception:
        hook = None
    mod = types.ModuleType("antenv.axon_hooks")
    mod._hook = hook
    mod.get_axon_ntff_profile_hook = lambda: mod._hook
    mod.set_axon_ntff_profile_hook = lambda h: setattr(mod, "_hook", h)
    sys.modules["antenv.axon_hooks"] = mod


def _prefix_ids(seq):
    h = np.cumsum(SEQ_A[None, :] * (seq % PRIME), axis=-1) % PRIME
    lengths = (seq != 0).sum(axis=-1, keepdims=True)
    pos = np.arange(seq.shape[-1], dtype=np.int64)[None, :]
    idx = np.minimum(pos, np.maximum(lengths - 1, 0))
    return np.take_along_axis(h, idx, axis=-1)


def _build_program(nsets):
    ng = nsets // SETS_PER_GATHER
    gw = GATHER_ROWS * 16  # free width of one gathered tile (8192)
    nc = bacc.Bacc()
    table = nc.declare_dram_parameter(
        "table", [ROWS_PER_CORE, ROW_W], mybir.dt.bfloat16, isOutput=False
    )
    idx_d = nc.declare_dram_parameter(
        "idx", [P, ng * (GATHER_ROWS // 16)], mybir.dt.int16, isOutput=False
    )
    rhs_d = nc.declare_dram_parameter(
        "rhs", [P, nsets * 64], mybir.dt.bfloat16, isOutput=False
    )
    out_d = nc.declare_dram_parameter(
        "out", [P, nsets * 64], mybir.dt.bfloat16, isOutput=True
    )

    with TileContext(nc) as tc:
        with (
            tc.tile_pool(name="misc", bufs=1) as misc,
            tc.tile_pool(name="gath", bufs=4) as gpool,
            tc.tile_pool(name="gat2", bufs=3) as g2pool,
            tc.tile_pool(name="rhs", bufs=3) as rpool,
            tc.tile_pool(name="ostg", bufs=4) as opool,
            tc.tile_pool(name="psum", bufs=4, space="PSUM") as pspool,
        ):
            nc.gpsimd.load_library(library_config.mlp)
            icols = GATHER_ROWS // 16
            idx_sb = misc.tile([P, ng * icols], mybir.dt.int16)
            nc.sync.dma_start(out=idx_sb[:, :], in_=idx_d[:, :])

            for g in range(ng):
                gt = gpool.tile([P, gw], mybir.dt.bfloat16, tag="g")
                gt3 = gt[:, :].rearrange("p (c i) -> p c i", i=GATHER_ROWS)
                nc.gpsimd.dma_gather(
                    out_ap=gt3,
                    in_ap=table[:, :],
                    idxs_ap=idx_sb[:, g * icols : (g + 1) * icols],
                    num_idxs=GATHER_ROWS,
                    num_idxs_reg=GATHER_ROWS,
                    elem_size=ROW_W,
                    transpose=True,
                )
                # repack (c, s, il) -> (s, c, il) so each set's stationary
                # is one contiguous 128-col free dim (BIR requires 2D APs)
                gt2 = g2pool.tile([P, gw], mybir.dt.bfloat16, tag="g2")
                nc.vector.tensor_copy(
                    out=gt2[:, :].rearrange(
                        "p (s c i) -> p s c i", c=16, s=SETS_PER_GATHER, i=8
                    ),
                    in_=gt[:, :].rearrange(
                        "p (c s i) -> p s c i", c=16, s=SETS_PER_GATHER, i=8
                    ),
                )
                rt = rpool.tile([P, SETS_PER_GATHER * 64], mybir.dt.bfloat16, tag="r")
                nc.sync.dma_start(
                    out=rt[:, :],
                    in_=rhs_d[:, g * SETS_PER_GATHER * 64 : (g + 1) * SETS_PER_GATHER * 64],
                )
                for blk in range(SETS_PER_GATHER // 8):
                    pb = pspool.tile([P, 512], mybir.dt.float32, tag="ps")
                    for k in range(8):
                        s8 = blk * 8 + k
                        lhsT = gt2[:, s8 * 128 : (s8 + 1) * 128]
                        nc.tensor.matmul(
                            pb[:, k * 64 : (k + 1) * 64],
                            lhsT,
                            rt[:, s8 * 64 : (s8 + 1) * 64],
                            start=True,
                            stop=True,
                        )
                    ob = opool.tile([P, 512], mybir.dt.bfloat16, tag="o")
                    # alternate drains between ScalarE and VectorE
                    if blk % 2 == 0:
                        nc.scalar.copy(out=ob[:, :], in_=pb[:, :])
                    else:
                        nc.vector.tensor_copy(out=ob[:, :], in_=pb[:, :])
                    s0 = g * SETS_PER_GATHER + blk * 8
                    nc.sync.dma_start(
                        out=out_d[:, s0 * 64 : (s0 + 8) * 64], in_=ob[:, :]
                    )
    nc.finalize()
    return nc


def _pack_sets(local_rows, row_of_inst_order):
    """Bin-pack rows (sorted by count desc) into sets of <=8 row-slots and
    <=32 instances. Returns (set_rows [nsets, 8] int16,
    inst_set/inst_slot/inst_il arrays aligned with the instance order implied
    by iterating rows in the sorted order)."""
    # local_rows: list of (row_id, count) sorted desc by count
    set_rows = []
    inst_set = []
    inst_slot = []
    inst_il = []
    cur_rows = []
    cur_n = 0

    def flush():
        nonlocal cur_rows, cur_n
        if cur_rows:
            set_rows.append(cur_rows + [0] * (ROWS_PER_SET - len(cur_rows)))
            cur_rows = []
            cur_n = 0

    for row_id, cnt in local_rows:
        left = cnt
        while left > 0:
            if len(cur_rows) >= ROWS_PER_SET or cur_n >= S_SLOTS:
                flush()
            take = min(left, S_SLOTS - cur_n)
            il = len(cur_rows)
            cur_rows.append(row_id)
            s = len(set_rows)  # current set index
            for _ in range(take):
                inst_set.append(s)
                inst_slot.append(cur_n)
                inst_il.append(il)
                cur_n += 1
            left -= take
    flush()
    return (
        np.asarray(set_rows, dtype=np.int16).reshape(-1, ROWS_PER_SET),
        np.asarray(inst_set, dtype=np.int64),
        np.asarray(inst_slot, dtype=np.int64),
        np.asarray(inst_il, dtype=np.int64),
    )


def kernel(input_sequence, t_representation, importance_weights, bucket_table):
    global LAST_RESULT
    input_sequence = np.asarray(input_sequence, dtype=np.int64)
    t_representation = np.asarray(t_representation, dtype=np.float32)
    importance_weights = np.asarray(importance_weights, dtype=np.float32)
    bucket_table = np.asarray(bucket_table, dtype=np.float32)

    B, D = input_sequence.shape
    npos = B * D

    ids = _prefix_ids(input_sequence)
    ids_f = ids.reshape(-1)
    w_all = importance_weights[ids_f % NUM_EMB]  # [npos, 2]
    t_flat = t_representation.reshape(npos, EMB)

    pos_arr = np.tile(np.arange(npos, dtype=np.int64), NUM_HASHES)
    h_arr = np.repeat(np.arange(NUM_HASHES, dtype=np.int64), npos)
    bucket_arr = np.concatenate(
        [((HASH_A[h] * ids_f + HASH_B[h]) % PRIME) % NUM_BUCKETS for h in range(NUM_HASHES)]
    )
    w_arr = np.concatenate([w_all[:, h] for h in range(NUM_HASHES)]).astype(np.float32)

    core_arr = bucket_arr // ROWS_PER_CORE
    local_arr = bucket_arr % ROWS_PER_CORE

    table_bf16 = np.ascontiguousarray(bucket_table.astype(ml_dtypes.bfloat16))

    # Per-core packing
    per_core = []
    for c in range(N_CORES):
        m = core_arr == c
        pos_c, h_c, loc_c, w_c = pos_arr[m], h_arr[m], local_arr[m], w_arr[m]
        # sort instances by local row; rows ordered by count desc
        uniq, inv, counts = np.unique(loc_c, return_inverse=True, return_counts=True)
        order_rows = np.argsort(-counts, kind="stable")  # indices into uniq
        # instance order: grouped by row in that row order
        rank_of_row = np.empty_like(order_rows)
        rank_of_row[order_rows] = np.arange(len(order_rows))
        inst_order = np.argsort(rank_of_row[inv], kind="stable")
        pos_c, h_c, w_c = pos_c[inst_order], h_c[inst_order], w_c[inst_order]
        local_rows = [(int(uniq[r]), int(counts[r])) for r in order_rows]
        set_rows, inst_set, inst_slot, inst_il = _pack_sets(local_rows, None)
        per_core.append(
            dict(pos=pos_c, h=h_c, w=w_c, set_rows=set_rows,
                 inst_set=inst_set, inst_slot=inst_slot, inst_il=inst_il)
        )

    nsets_max = max(pc["set_rows"].shape[0] for pc in per_core)
    NSETS = int(math.ceil(nsets_max / SETS_PER_GATHER) * SETS_PER_GATHER)
    NG = NSETS // SETS_PER_GATHER

    key = NSETS
    if key not in _PROGRAM_CACHE:
        _PROGRAM_CACHE[key] = _build_program(NSETS)
    nc = _PROGRAM_CACHE[key]

    in_maps = []
    for c in range(N_CORES):
        pc = per_core[c]
        ns = pc["set_rows"].shape[0]
        rows_pad = np.zeros((NSETS, ROWS_PER_SET), dtype=np.int16)
        rows_pad[:ns] = pc["set_rows"]
        # idx tile [128, NG*icols]: gather g, col s, partition 16k+p holds
        # row index i = s*16 + p of that gather's GATHER_ROWS-row list.
        icols = GATHER_ROWS // 16
        L = rows_pad.reshape(NG, GATHER_ROWS)
        idx_tile = np.zeros((P, NG * icols), dtype=np.int16)
        for g in range(NG):
            wrap = L[g].reshape(icols, 16).T  # [p=16, s=icols]
            idx_tile[:, g * icols : (g + 1) * icols] = np.tile(wrap, (8, 1))

        # rhs [128, NSETS*64]
        tvw = (t_flat[pc["pos"]] * pc["w"][:, None]).astype(ml_dtypes.bfloat16)  # [n, 64]
        rhs = np.zeros((P, NSETS * 64), dtype=ml_dtypes.bfloat16)
        col0 = pc["inst_set"] * 64 + pc["inst_slot"]
        col1 = pc["inst_set"] * 64 + 32 + pc["inst_slot"]
        rhs[0:64, col0] = tvw.T
        rhs[64:128, col1] = tvw.T

        in_maps.append(
            {
                "table": table_bf16[c * ROWS_PER_CORE : (c + 1) * ROWS_PER_CORE],
                "idx": idx_tile,
                "rhs": rhs,
            }
        )

    if TRACE:
        _ensure_ntff_hook()
    res = run_bass_kernel_spmd(nc, in_maps, list(range(N_CORES)), trace=TRACE)
    LAST_RESULT = res

    out2 = np.zeros((NUM_HASHES, npos, N_ARY), dtype=np.float32)
    c_idx = np.arange(16)
    for c in range(N_CORES):
        pc = per_core[c]
        out_t = np.asarray(res.results[c]["out"]).astype(np.float32)  # [128, NSETS*64]
        # m = c*8 + il  (c-major AP: free dims (c:16, il:8), il innermost)
        part = c_idx[None, :] * 8 + pc["inst_il"][:, None]  # [n, 16]
        colA = (pc["inst_set"] * 64 + pc["inst_slot"])[:, None]        # j=0 -> a=2c
        colB = (pc["inst_set"] * 64 + 32 + pc["inst_slot"])[:, None]   # j=1 -> a=2c+1
        vA = out_t[part, colA]  # [n, 16]
        vB = out_t[part, colB]  # [n, 16]
        logits = np.empty((part.shape[0], N_ARY), dtype=np.float32)
        logits[:, 0::2] = vA
        logits[:, 1::2] = vB
        out2[pc["h"], pc["pos"]] = logits
    out = out2.sum(axis=0).reshape(B, D, N_ARY)
    return out
